# revision 13
# baseline (speedup 1.0000x reference)
"""Masked-softmax attention (B=8, NQ=1024, S=2048, D=512) on 8 TRN2 NeuronCores.

Data-parallel: one batch element per core. Per-core math (all in transposed
layout so no on-chip transposes are needed):

    S^T[s, q]  = sum_d K^T[d, s-tile] . Q^T[d, q]          (PE, fp32r)
    E^T        = exp(S^T - 100)                             (ACT, PSUM->SBUF)
    P^T        = E^T * mask^T                               (DVE, bf16 mask)
    O^T[d, q] += V[s-tile, d-tile]^T-as-lhsT @ P^T          (PE accumulate)
    R[*, q]   += ones^T @ P^T  (row sums, broadcast to all partitions by PE)
    O^T       *= 1/R                                        (DVE)

The softmax max-subtraction is replaced by a constant offset (-100): scores
are N(0, sqrt(512)) so exp(S-100) neither overflows nor all-underflows; the
offset cancels exactly in the renormalization (as does the softmax Z).

Host side only shards + transposes + casts the mask to bf16; all FLOPs run
on device.
"""

import numpy as np
import ml_dtypes

import concourse.bass as bass
import concourse.bass_isa as bass_isa
import concourse.mybir as mybir
import concourse.tile as tile
from concourse import bacc
from concourse.bass_utils import run_bass_kernel_spmd

B, NQ, S, D = 8, 1024, 2048, 512
NCORES = 8

P = 128              # partition tile
QCH = 512            # q chunk (matmul free dim / PSUM bank)
N_QCH = NQ // QCH    # 2
N_ST = S // P        # 16 s-tiles
N_DT = D // P        # 4 d-tiles
SGRP = 4             # s-tiles per DMA group
N_SG = N_ST // SGRP  # 4

F32 = mybir.dt.float32
F32R = mybir.dt.float32r
F16 = mybir.dt.float16
BF16 = mybir.dt.bfloat16
EXP_OFFSET = -100.0


def build_nc():
    nc = bacc.Bacc("TRN2", target_bir_lowering=False, debug=False,
                   num_devices=NCORES)
    qT = nc.declare_dram_parameter("qT", [D, NQ], F16, isOutput=False)
    kT = nc.declare_dram_parameter("kT", [D, S], F16, isOutput=False)
    v = nc.declare_dram_parameter("v", [S, D], BF16, isOutput=False)
    mT = nc.declare_dram_parameter("mT", [S, NQ], BF16, isOutput=False)
    oT = nc.declare_dram_parameter("oT", [D, NQ], F32, isOutput=True)

    with tile.TileContext(nc) as tc:
        with (
            tc.tile_pool(name="consts", bufs=1) as consts,
            tc.tile_pool(name="qt", bufs=1) as qt_pool,
            tc.tile_pool(name="kt", bufs=1) as kt_pool,
            tc.tile_pool(name="vp", bufs=1) as v_pool,
            tc.tile_pool(name="mp", bufs=1) as m_pool,
            tc.tile_pool(name="e", bufs=3) as e_pool,
            tc.tile_pool(name="p", bufs=4) as p_pool,
            tc.tile_pool(name="osb", bufs=4) as o_pool,
            tc.tile_pool(name="rec", bufs=2) as r_pool,
            tc.tile_pool(name="ps_s", bufs=4, space="PSUM") as ps_s,
            tc.tile_pool(name="ps_o", bufs=4, space="PSUM") as ps_o,
        ):
            ones_f32 = consts.tile([P, P], F32)
            nc.vector.memset(ones_f32[:, :], 1.0)
            ones_t = consts.tile([P, P], F32R)
            nc.vector.tensor_copy(ones_t[:, :], ones_f32[:, :])
            warm_t = consts.tile([P, P], BF16)
            nc.gpsimd.memset(warm_t[:, :], 0.0)
            bias_t = consts.tile([P, 1], F32)
            nc.vector.memset(bias_t[:, :], EXP_OFFSET)

            # Resident inputs. First groups are small so the minimal compute
            # set (qt0 + kt/m/v of group 0) lands as the PE warmup drains.
            GROUPS = [(0, 2), (2, 2), (4, 2), (6, 2), (8, 4), (12, 4)]
            tile2grp = {}
            for gi, (gs, gn) in enumerate(GROUPS):
                for t in range(gn):
                    tile2grp[gs + t] = (gi, t)
            qt_sb = [qt_pool.tile([P, N_DT, QCH], F16, tag=f"qt{c}", name=f"qt{c}")
                     for c in range(N_QCH)]
            kt_sb = [kt_pool.tile([P, N_DT, gn * P], F16, tag=f"kt{g}", name=f"kt{g}")
                     for g, (gs, gn) in enumerate(GROUPS)]
            v_sb = [v_pool.tile([P, gn, D], BF16, tag=f"v{g}", name=f"v{g}")
                    for g, (gs, gn) in enumerate(GROUPS)]
            m_sb = [m_pool.tile([P, gn, NQ], BF16, tag=f"m{g}", name=f"m{g}")
                    for g, (gs, gn) in enumerate(GROUPS)]

            for w in range(44):
                wp = ps_s.tile([P, P], F32, name="warm_psum", tag="st")
                nc.tensor.matmul(wp[:, :], lhsT=warm_t[:, :], rhs=warm_t[:, :],
                                 start=True, stop=True)

            nc.sync.dma_start(
                out=qt_sb[0][:, :, :],
                in_=qT[:, 0:QCH].rearrange("(t p) q -> p t q", p=P))
            for g, (gs, gn) in enumerate(GROUPS):
                s0 = gs * P
                s1 = (gs + gn) * P
                nc.sync.dma_start(
                    out=kt_sb[g][:, :, :],
                    in_=kT[:, s0:s1].rearrange("(t p) s -> p t s", p=P))
                nc.sync.dma_start(
                    out=m_sb[g][:, :, :],
                    in_=mT[s0:s1, :].rearrange("(t p) q -> p t q", p=P))
                nc.sync.dma_start(
                    out=v_sb[g][:, :, :],
                    in_=v[s0:s1, :].rearrange("(t p) d -> p t d", p=P))
            nc.sync.dma_start(
                out=qt_sb[1][:, :, :],
                in_=qT[:, QCH:NQ].rearrange("(t p) q -> p t q", p=P))

            LAG = 2
            for c in range(N_QCH):
                o_psum = [ps_o.tile([P, QCH], F32, name="o_psum") for _ in range(N_DT)]
                r_acc = r_pool.tile([P, QCH], F32R, name="r_acc", tag="r_acc")
                p_tiles = {}
                # Software pipeline: matmul2 for s-tile (step-LAG) is emitted
                # after matmul1 for s-tile step, so the PE stream always has
                # independent work while exp/mask of the newest tile run.
                for step in range(N_ST + LAG):
                    if step < N_ST:
                        si = step
                        g, sl = tile2grp[si]
                        st = ps_s.tile([P, QCH], F32)
                        for di in range(N_DT):
                            nc.tensor.matmul(st[:, :],
                                             lhsT=kt_sb[g][:, di, sl * P:(sl + 1) * P],
                                             rhs=qt_sb[c][:, di, :],
                                             start=(di == 0), stop=(di == N_DT - 1))
                        e_t = e_pool.tile([P, QCH], BF16)
                        nc.scalar.activation(out=e_t[:, :], in_=st[:, :],
                                             func=mybir.ActivationFunctionType.Exp,
                                             bias=bias_t[:, 0:1], scale=1.0)
                        p_t = p_pool.tile([P, QCH], BF16)
                        nc.vector.tensor_mul(p_t[:, :], e_t[:, :],
                                             m_sb[g][:, sl, c * QCH:(c + 1) * QCH])
                        # Row-sum partial accumulation on DVE (f32r so a
                        # single PE ones-matmul can finish the reduction).
                        if si == 0:
                            nc.vector.tensor_copy(r_acc[:, :], p_t[:, :])
                        else:
                            nc.vector.tensor_add(r_acc[:, :], r_acc[:, :],
                                                 p_t[:, :])
                        p_tiles[si] = p_t
                    if step == N_ST:
                        # Partition-sum of r_acc via one ones-matmul; result
                        # is replicated across all 128 partitions.
                        r_psum = ps_s.tile([P, QCH], F32, name="r_psum",
                                           tag="st")
                        nc.tensor.matmul(r_psum[:, :], lhsT=ones_t[:, :],
                                         rhs=r_acc[:, :],
                                         start=True, stop=True)
                        recip = r_pool.tile([P, QCH], F32)
                        nc.vector.reciprocal_approx_fast(recip[:, :],
                                                         r_psum[:, :])
                    if step >= LAG:
                        sj = step - LAG
                        gj, slj = tile2grp[sj]
                        p_r = p_tiles.pop(sj)[:, :]
                        for di in range(N_DT):
                            nc.tensor.matmul(o_psum[di][:, :],
                                             lhsT=v_sb[gj][:, slj, di * P:(di + 1) * P],
                                             rhs=p_r,
                                             start=(sj == 0), stop=(sj == N_ST - 1))
                for di in range(N_DT):
                    o_sb = o_pool.tile([P, QCH], F32)
                    nc.vector.tensor_mul(o_sb[:, :], o_psum[di][:, :],
                                         recip[:, :])
                    nc.sync.dma_start(
                        out=oT[di * P:(di + 1) * P, c * QCH:(c + 1) * QCH],
                        in_=o_sb[:, :])
    nc.compile()
    return nc


_NC = None


def _get_nc():
    global _NC
    if _NC is None:
        _NC = build_nc()
    return _NC


def kernel(queries, keys, values, mask):
    nc = _get_nc()
    in_maps = []
    for i in range(NCORES):
        in_maps.append({
            "qT": np.ascontiguousarray(queries[i].T, dtype=np.float16),
            "kT": np.ascontiguousarray(keys[i].T, dtype=np.float16),
            "v": np.ascontiguousarray(values[i]).astype(ml_dtypes.bfloat16),
            "mT": np.ascontiguousarray(mask[i].T).astype(ml_dtypes.bfloat16),
        })
    res = run_bass_kernel_spmd(nc, in_maps, core_ids=list(range(NCORES)))
    out = np.stack([res.results[i]["oT"].T for i in range(NCORES)])
    return np.ascontiguousarray(out, dtype=np.float32)


# revision 14
# speedup vs baseline: 1.0423x; 1.0423x over previous
"""Masked-softmax attention (B=8, NQ=1024, S=2048, D=512) on 8 TRN2 NeuronCores.

Data-parallel: one batch element per core. Per-core math (all in transposed
layout so no on-chip transposes are needed):

    S^T[s, q]  = sum_d K^T[d, s-tile] . Q^T[d, q]          (PE, fp32r)
    E^T        = exp(S^T - 100)                             (ACT, PSUM->SBUF)
    P^T        = E^T * mask^T                               (DVE, bf16 mask)
    O^T[d, q] += V[s-tile, d-tile]^T-as-lhsT @ P^T          (PE accumulate)
    R[*, q]   += ones^T @ P^T  (row sums, broadcast to all partitions by PE)
    O^T       *= 1/R                                        (DVE)

The softmax max-subtraction is replaced by a constant offset (-100): scores
are N(0, sqrt(512)) so exp(S-100) neither overflows nor all-underflows; the
offset cancels exactly in the renormalization (as does the softmax Z).

Host side only shards + transposes + casts the mask to bf16; all FLOPs run
on device.
"""

import numpy as np
import ml_dtypes

import concourse.bass as bass
import concourse.bass_isa as bass_isa
import concourse.mybir as mybir
import concourse.tile as tile
from concourse import bacc
from concourse.bass_utils import run_bass_kernel_spmd

B, NQ, S, D = 8, 1024, 2048, 512
NCORES = 8

P = 128              # partition tile
QCH = 512            # q chunk (matmul free dim / PSUM bank)
N_QCH = NQ // QCH    # 2
N_ST = S // P        # 16 s-tiles
N_DT = D // P        # 4 d-tiles
SGRP = 4             # s-tiles per DMA group
N_SG = N_ST // SGRP  # 4

F32 = mybir.dt.float32
F32R = mybir.dt.float32r
F16 = mybir.dt.float16
BF16 = mybir.dt.bfloat16
U8 = mybir.dt.uint8
EXP_OFFSET = -100.0


def build_nc():
    nc = bacc.Bacc("TRN2", target_bir_lowering=False, debug=False,
                   num_devices=NCORES)
    qT = nc.declare_dram_parameter("qT", [D, NQ], F16, isOutput=False)
    kT = nc.declare_dram_parameter("kT", [D, S], F16, isOutput=False)
    v = nc.declare_dram_parameter("v", [S, D], BF16, isOutput=False)
    mT = nc.declare_dram_parameter("mT", [S, NQ], U8, isOutput=False)
    oT = nc.declare_dram_parameter("oT", [D, NQ], F32, isOutput=True)

    with tile.TileContext(nc) as tc:
        with (
            tc.tile_pool(name="consts", bufs=1) as consts,
            tc.tile_pool(name="qt", bufs=1) as qt_pool,
            tc.tile_pool(name="kt", bufs=1) as kt_pool,
            tc.tile_pool(name="vp", bufs=1) as v_pool,
            tc.tile_pool(name="mp", bufs=1) as m_pool,
            tc.tile_pool(name="e", bufs=4) as e_pool,
            tc.tile_pool(name="p", bufs=5) as p_pool,
            tc.tile_pool(name="osb", bufs=4) as o_pool,
            tc.tile_pool(name="rec", bufs=2) as r_pool,
            tc.tile_pool(name="ps_s", bufs=4, space="PSUM") as ps_s,
            tc.tile_pool(name="ps_o", bufs=4, space="PSUM") as ps_o,
        ):
            ones_f32 = consts.tile([P, P], F32)
            nc.vector.memset(ones_f32[:, :], 1.0)
            ones_t = consts.tile([P, P], F32R)
            nc.vector.tensor_copy(ones_t[:, :], ones_f32[:, :])
            warm_t = consts.tile([P, P], BF16)
            nc.gpsimd.memset(warm_t[:, :], 0.0)
            bias_t = consts.tile([P, 1], F32)
            nc.vector.memset(bias_t[:, :], EXP_OFFSET)

            # Resident inputs. First groups are small so the minimal compute
            # set (qt0 + kt/m/v of group 0) lands as the PE warmup drains.
            GROUPS = [(0, 2), (2, 2), (4, 2), (6, 2), (8, 4), (12, 4)]
            tile2grp = {}
            for gi, (gs, gn) in enumerate(GROUPS):
                for t in range(gn):
                    tile2grp[gs + t] = (gi, t)
            qt_sb = [qt_pool.tile([P, N_DT, QCH], F16, tag=f"qt{c}", name=f"qt{c}")
                     for c in range(N_QCH)]
            kt_sb = [kt_pool.tile([P, N_DT, gn * P], F16, tag=f"kt{g}", name=f"kt{g}")
                     for g, (gs, gn) in enumerate(GROUPS)]
            v_sb = [v_pool.tile([P, gn, D], BF16, tag=f"v{g}", name=f"v{g}")
                    for g, (gs, gn) in enumerate(GROUPS)]
            m_sb = [m_pool.tile([P, gn, NQ], U8, tag=f"m{g}", name=f"m{g}")
                    for g, (gs, gn) in enumerate(GROUPS)]

            for w in range(44):
                wp = ps_s.tile([P, P], F32, name="warm_psum", tag="st")
                nc.tensor.matmul(wp[:, :], lhsT=warm_t[:, :], rhs=warm_t[:, :],
                                 start=True, stop=True)

            nc.sync.dma_start(
                out=qt_sb[0][:, :, :],
                in_=qT[:, 0:QCH].rearrange("(t p) q -> p t q", p=P))
            for g, (gs, gn) in enumerate(GROUPS):
                s0 = gs * P
                s1 = (gs + gn) * P
                nc.sync.dma_start(
                    out=kt_sb[g][:, :, :],
                    in_=kT[:, s0:s1].rearrange("(t p) s -> p t s", p=P))
                nc.sync.dma_start(
                    out=m_sb[g][:, :, :],
                    in_=mT[s0:s1, :].rearrange("(t p) q -> p t q", p=P))
                nc.sync.dma_start(
                    out=v_sb[g][:, :, :],
                    in_=v[s0:s1, :].rearrange("(t p) d -> p t d", p=P))
            nc.sync.dma_start(
                out=qt_sb[1][:, :, :],
                in_=qT[:, QCH:NQ].rearrange("(t p) q -> p t q", p=P))

            LAG = 3
            for c in range(N_QCH):
                o_psum = [ps_o.tile([P, QCH], F32, name="o_psum") for _ in range(N_DT)]
                r_acc = r_pool.tile([P, QCH], F32R, name="r_acc", tag="r_acc")
                p_tiles = {}
                # Software pipeline: matmul2 for s-tile (step-LAG) is emitted
                # after matmul1 for s-tile step, so the PE stream always has
                # independent work while exp/mask of the newest tile run.
                for step in range(N_ST + LAG):
                    if step < N_ST:
                        si = step
                        g, sl = tile2grp[si]
                        st = ps_s.tile([P, QCH], F32)
                        for di in range(N_DT):
                            nc.tensor.matmul(st[:, :],
                                             lhsT=kt_sb[g][:, di, sl * P:(sl + 1) * P],
                                             rhs=qt_sb[c][:, di, :],
                                             start=(di == 0), stop=(di == N_DT - 1))
                        e_t = e_pool.tile([P, QCH], BF16)
                        nc.scalar.activation(out=e_t[:, :], in_=st[:, :],
                                             func=mybir.ActivationFunctionType.Exp,
                                             bias=bias_t[:, 0:1], scale=1.0)
                        p_t = p_pool.tile([P, QCH], BF16)
                        nc.vector.tensor_mul(p_t[:, :], e_t[:, :],
                                             m_sb[g][:, sl, c * QCH:(c + 1) * QCH])
                        # Row-sum partial accumulation on DVE (f32r so a
                        # single PE ones-matmul can finish the reduction).
                        if si == 0:
                            nc.vector.tensor_copy(r_acc[:, :], p_t[:, :])
                        else:
                            nc.vector.tensor_add(r_acc[:, :], r_acc[:, :],
                                                 p_t[:, :])
                        p_tiles[si] = p_t
                    if step == N_ST:
                        # Partition-sum of r_acc via one ones-matmul; result
                        # is replicated across all 128 partitions.
                        r_psum = ps_s.tile([P, QCH], F32, name="r_psum",
                                           tag="st")
                        nc.tensor.matmul(r_psum[:, :], lhsT=ones_t[:, :],
                                         rhs=r_acc[:, :],
                                         start=True, stop=True)
                        recip = r_pool.tile([P, QCH], F32)
                        nc.vector.reciprocal_approx_fast(recip[:, :],
                                                         r_psum[:, :])
                    if step >= LAG:
                        sj = step - LAG
                        gj, slj = tile2grp[sj]
                        p_r = p_tiles.pop(sj)[:, :]
                        for di in range(N_DT):
                            nc.tensor.matmul(o_psum[di][:, :],
                                             lhsT=v_sb[gj][:, slj, di * P:(di + 1) * P],
                                             rhs=p_r,
                                             start=(sj == 0), stop=(sj == N_ST - 1))
                for di in range(N_DT):
                    o_sb = o_pool.tile([P, QCH], F32)
                    nc.vector.tensor_mul(o_sb[:, :], o_psum[di][:, :],
                                         recip[:, :])
                    nc.sync.dma_start(
                        out=oT[di * P:(di + 1) * P, c * QCH:(c + 1) * QCH],
                        in_=o_sb[:, :])
    nc.compile()
    return nc


_NC = None


def _get_nc():
    global _NC
    if _NC is None:
        _NC = build_nc()
    return _NC


def kernel(queries, keys, values, mask):
    nc = _get_nc()
    in_maps = []
    for i in range(NCORES):
        in_maps.append({
            "qT": np.ascontiguousarray(queries[i].T, dtype=np.float16),
            "kT": np.ascontiguousarray(keys[i].T, dtype=np.float16),
            "v": np.ascontiguousarray(values[i]).astype(ml_dtypes.bfloat16),
            "mT": np.ascontiguousarray(mask[i].T).astype(np.uint8),
        })
    res = run_bass_kernel_spmd(nc, in_maps, core_ids=list(range(NCORES)))
    out = np.stack([res.results[i]["oT"].T for i in range(NCORES)])
    return np.ascontiguousarray(out, dtype=np.float32)
